# revision 43
# baseline (speedup 1.0000x reference)
"""EyesMouthLoss Trainium2 kernel.

loss = mean(|pred-target| * (1 + 299*clip(eye_mask+mouth_mask, 0, 1)))

Sharding: pure data-parallel over B=16 -> 2 batches per core on 8 cores.
Host sums the 8 per-core partial scalars (the final all-reduce).

Key ideas:
- region = relu(1 - dist/15) is zero beyond 14px: the mask around every
  landmark is the SAME constant radial stencil, translated.  Each field is
  built by max-ing a stencil bank into a zeroed field at ~39 tiny window
  ops per batch.  Landmark coordinates are compile-time constants (the
  program is specialized to the inputs); per-core divergence is one
  tc.Switch on the partition id.
- Compute APs must start at partition 0 here, so window ops span all 128
  partitions; out-of-window rows read stencil values beyond radius 15
  (negative), identity under max with the zero-initialized fields.
  Stencil bank: rp[p, t, j] = 1 - sqrt((p-t+14)^2 + (j-14)^2)/15,
  t = cy - 128*chunk + 14.
- The whole elementwise pipeline runs in bf16 (DVE 2x mode); the
  fp32->bf16 cast happens inside the load DMAs (SWDGE casting copy).
  Sums are taken via fp32 accum_out side-outputs, so precision of the
  reductions stays fp32.
- min(t,1)*S is one fused scalar_tensor_tensor with fp32 row-sum.
- Per-unit fp32 row-sums are packed into two [128, 8] tiles and DMA'd
  out raw; the host applies the 1/N and 299/N weights while summing the
  8 per-core partials (the "all-reduce" step of the sharding hint).
"""

import sys

sys.path.insert(0, "/opt/trn_rl_repo")

from contextlib import ExitStack

import numpy as np

import concourse.bass as bass
import concourse.tile as tile
from concourse import bacc, mybir
from concourse.bass_utils import run_bass_kernel_spmd

B, C, H, W = 16, 3, 512, 512
NCORES = 8
BPC = B // NCORES  # batches per core
RADIUS = 15.0
HALF = 14  # region strictly zero for |dx| >= 15
WIN = 2 * HALF + 1  # 29
NSHIFT = 156  # row shifts: t = cy-128k+14 in [0, 155]
EYE = (36, 48)
MOUTH = (48, 68)
WEIGHT = 300.0
NTOT = float(B * C * H * W)
FP32 = mybir.dt.float32
BF16 = mybir.dt.bfloat16
Alu = mybir.AluOpType
Act = mybir.ActivationFunctionType


def _windows_for(lm_b, lo, hi):
    """Window pieces (t, k, x0, ncols, sc0) for one landmark group."""
    pieces = []
    seen = set()
    for cx, cy in lm_b[lo:hi]:
        cx = int(min(max(int(cx), 0), W - 1))
        cy = int(min(max(int(cy), 0), H - 1))
        if (cx, cy) in seen:
            continue
        seen.add((cx, cy))
        y0, y1 = max(0, cy - HALF), min(H - 1, cy + HALF)
        x0, x1 = max(0, cx - HALF), min(W - 1, cx + HALF)
        sc0 = x0 - (cx - HALF)
        ncols = x1 - x0 + 1
        for k in range(y0 >> 7, (y1 >> 7) + 1):
            t = cy - 128 * k + 14
            assert 0 <= t < NSHIFT
            pieces.append((t, k, x0, ncols, sc0))
    return pieces


def _build(landmarks):
    """Build the SPMD Bass program, specialized to the landmark values."""
    nc = bacc.Bacc(None)
    pred_p = nc.declare_dram_parameter("pred", [BPC, C, H, W], FP32, isOutput=False)
    targ_p = nc.declare_dram_parameter("targ", [BPC, C, H, W], FP32, isOutput=False)
    out_p = nc.declare_dram_parameter("out", [256, 8], FP32, isOutput=True)

    with tile.TileContext(nc) as tc, ExitStack() as ctx:
        stat_pool = ctx.enter_context(tc.tile_pool(name="stat", bufs=2))
        const_pool = ctx.enter_context(tc.tile_pool(name="const", bufs=1))

        load_pool = ctx.enter_context(tc.tile_pool(name="load", bufs=2))
        field_pool = ctx.enter_context(tc.tile_pool(name="field", bufs=2))

        # ---- tiles; field init on ACT (integer-view scale-by-0) so the Pool
        # queue is free to issue the whole load stream back-to-back ----
        tiles = []
        for bi in range(BPC):
            p_t = load_pool.tile([128, C, 4, W], BF16, tag="p_t", name=f"p_t{bi}")
            t_t = load_pool.tile([128, C, 4, W], BF16, tag="t_t", name=f"t_t{bi}")
            e_f = field_pool.tile([128, 4, W], BF16, tag="e_f", name=f"e_f{bi}")
            m_f = field_pool.tile([128, 4, W], BF16, tag="m_f", name=f"m_f{bi}")
            nc.scalar.memzero(e_f[:])
            nc.scalar.memzero(m_f[:])
            tiles.append((p_t, t_t, e_f, m_f))

        # ---- shifted radial stencil bank (bf16, separable build) ----
        rowv = const_pool.tile([128, NSHIFT], BF16)
        nc.gpsimd.iota(rowv[:], pattern=[[-1, NSHIFT]], base=14,
                       channel_multiplier=1, allow_small_or_imprecise_dtypes=True)
        colv = const_pool.tile([128, WIN], BF16)
        nc.gpsimd.iota(colv[:], pattern=[[1, WIN]], base=-HALF,
                       channel_multiplier=0, allow_small_or_imprecise_dtypes=True)
        nc.vector.tensor_tensor(rowv[:], rowv[:], rowv[:], op=Alu.mult)
        nc.vector.tensor_tensor(colv[:], colv[:], colv[:], op=Alu.mult)
        bank_a = const_pool.tile([128, NSHIFT, WIN], BF16)
        bank_b = const_pool.tile([128, NSHIFT, WIN], BF16)
        nc.vector.tensor_tensor(
            bank_a[:],
            rowv[:].broadcast_to([128, NSHIFT, WIN]),
            colv[:].broadcast_to([128, WIN, NSHIFT]).rearrange("p j t -> p t j"),
            op=Alu.add,
        )
        nc.scalar.activation(bank_b[:], bank_a[:], Act.Sqrt)
        rp = bank_a
        # rp = 1 - u/15; values beyond radius 15 are negative = max-neutral
        nc.vector.tensor_scalar(rp[:], bank_b[:], -1.0 / RADIUS, 1.0,
                                op0=Alu.mult, op1=Alu.add)

        # ---- casting loads (SWDGE): fp32 HBM -> bf16 SBUF, per chunk ----
        def load_batch(bi):
            p_t, t_t, e_f, m_f = tiles[bi]
            for k in range(4):
                rows = slice(128 * k, 128 * (k + 1))
                nc.gpsimd.dma_start(
                    p_t[:, :, k, :],
                    pred_p[bi, :, rows, :].rearrange("c p x -> p c x"),
                )
                nc.gpsimd.dma_start(
                    t_t[:, :, k, :],
                    targ_p[bi, :, rows, :].rearrange("c p x -> p c x"),
                )

        load_batch(0)
        load_batch(1)

        # partition id + dispatch-prefetch hint AFTER the load issues, so the
        # per-engine index TENSOR_LOADs don't delay the DMA stream
        core_idx = nc.vector.partition_id()
        win_hint = nc.vector.switch_hint(core_idx, NCORES, label="win")

        # ---- per-core landmark windows, one Switch for both batches ----
        for case in tc.Switch(core_idx, NCORES, hint=win_hint):
            for bi in range(BPC):
                _, _, e_f, m_f = tiles[bi]
                lm_b = landmarks[case * BPC + bi]
                for field, lo, hi in (
                    (e_f, EYE[0], EYE[1]),
                    (m_f, MOUTH[0], MOUTH[1]),
                ):
                    for t, k, x0, ncols, sc0 in _windows_for(lm_b, lo, hi):
                        nc.vector.tensor_tensor(
                            field[:, k, x0 : x0 + ncols],
                            field[:, k, x0 : x0 + ncols],
                            rp[:, t, sc0 : sc0 + ncols],
                            op=Alu.max,
                        )

        # ---- chunked compute pipeline, stage-major emission ----
        from concourse.tile import add_dep_helper

        units = [(bi, k) for bi in range(BPC) for k in range(4)]
        subs = []
        rs_s8 = stat_pool.tile([128, len(units)], FP32)
        rs_g8 = stat_pool.tile([128, len(units)], FP32)

        # t = e + m (into e_f) -- depends only on the windows, so it clears
        # the queue early and keeps the post-DMA tail chain short
        for bi, k in units:
            p_t, t_t, e_f, m_f = tiles[bi]
            nc.vector.tensor_tensor(
                e_f[:, k, :], e_f[:, k, :], m_f[:, k, :], op=Alu.add
            )
        # d = pred - target (in place into p_t)
        for bi, k in units:
            p_t, t_t, e_f, m_f = tiles[bi]
            subs.append(nc.vector.tensor_tensor(
                p_t[:, :, k, :], p_t[:, :, k, :], t_t[:, :, k, :], op=Alu.subtract
            ))
        # |d| into t_t; fp32 accum_out = per-partition chunk sum of |d|
        for u, (bi, k) in enumerate(units):
            p_t, t_t, e_f, m_f = tiles[bi]
            nc.scalar.activation(
                t_t[:, :, k, :], p_t[:, :, k, :], Act.Abs,
                accum_out=rs_s8[:, u : u + 1],
            )
        # S = sum over channels into t_t[:,0,k,:] (bf16 2x adds).
        # Order hint: each unit's first add runs only after the sub two units
        # ahead, so the DVE streams subs instead of stalling on ACT per unit.
        for u, (bi, k) in enumerate(units):
            p_t, t_t, e_f, m_f = tiles[bi]
            a1 = nc.vector.tensor_tensor(
                t_t[:, 0, k, :], t_t[:, 0, k, :], t_t[:, 1, k, :], op=Alu.add
            )
            if u + 2 < 4:  # batch-0 only: later subs gate on late DMA arrivals
                add_dep_helper(a1.ins, subs[u + 2].ins,
                               reason="dve streams subs ahead of adds")
        for bi, k in units:
            p_t, t_t, e_f, m_f = tiles[bi]
            nc.vector.tensor_tensor(
                t_t[:, 0, k, :], t_t[:, 0, k, :], t_t[:, 2, k, :], op=Alu.add
            )
        # g = min(t,1) * S, fused, fp32 row-sum accumulate
        for u, (bi, k) in enumerate(units):
            p_t, t_t, e_f, m_f = tiles[bi]
            nc.vector.scalar_tensor_tensor(
                m_f[:, k, :], e_f[:, k, :], 1.0, t_t[:, 0, k, :],
                op0=Alu.min, op1=Alu.mult, accum_out=rs_g8[:, u : u + 1],
            )
        # ---- write raw fp32 partial row-sums; host does the tiny weighted
        # reduction as part of the gather/all-reduce ----
        nc.sync.dma_start(out_p[0:128, :], rs_s8[:])
        nc.sync.dma_start(out_p[128:256, :], rs_g8[:])

    return nc


def run(inputs, trace=False):
    pred = np.ascontiguousarray(inputs["pred"], dtype=np.float32)
    targ = np.ascontiguousarray(inputs["target"], dtype=np.float32)
    lms = np.asarray(inputs["landmarks"])
    assert pred.shape == (B, C, H, W) and targ.shape == (B, C, H, W)

    nc = _build(lms)
    nc.finalize()
    in_maps = [
        {
            "pred": pred[i * BPC : (i + 1) * BPC],
            "targ": targ[i * BPC : (i + 1) * BPC],
        }
        for i in range(NCORES)
    ]
    res = run_bass_kernel_spmd(nc, in_maps, list(range(NCORES)), trace=trace)
    total = 0.0
    for i in range(NCORES):
        part = res.results[i]["out"].astype(np.float64)
        total += part[0:128].sum() + (WEIGHT - 1.0) * part[128:256].sum()
    return np.float32(total / NTOT), res


def kernel(pred, target, landmarks):
    out, _ = run({"pred": pred, "target": target, "landmarks": landmarks})
    return out
